# revision 11
# baseline (speedup 1.0000x reference)
"""Trainium2 Bass kernel: single transformer decoder block
(B=4, T=1024, C=1024, H=16 heads, head_dim=64, FFN hidden=4096).

Sharding: sequence-parallel across 8 NeuronCores = 4 batches x 2
causally-balanced token halves.  No collectives: each core computes
LN1 + K/V projections for all 1024 tokens of its batch, and
attention + FFN for its own 512 tokens.  The host shards/permutes on
the way in and gathers/unpermutes on the way out.

On-chip everything runs in a transposed layout [C on partitions,
tokens on free dim]:
  - LN mean/var via ones-matmul partition reductions (bf16 operands,
    fp32 PSUM) + K=1 broadcast matmuls.
  - Scores are computed transposed (S^T = K_h^T-stationary x Q_h^T);
    causal masking is a multiplicative bf16 0/1 mask on exp(S^T); the
    softmax denominator comes from a ones-matmul accumulated alongside
    the A@V matmul.  No PE transposes, no max-subtraction (logits are
    bounded ~|0.8|).
  - Matmuls in bf16 with fp32 PSUM accumulation (rel err ~2e-3).
  - Every PSUM accumulation chain owns a whole bank (start=True resets
    accumulation state bank-wide, verified on HW).

Token permutation makes the kernel uniform across cores: own token
tiles (sorted by descending causal span) sit at positions 0..3,
partner tiles at 4..7, so even/odd cores share one instruction
stream; per-core causal differences live in the mask01 tensor.

Single long-lived SBUF/PSUM pools with tag-chained slot reuse: no
mid-kernel pool releases (each release is an all-engine barrier).
"""

import numpy as np
import ml_dtypes

B, T, C, H = 4, 1024, 1024, 16
HD = 64
FF = 4096
P = 128
NT = 8  # token tiles == C tiles
OWN = 512
N_CORES = 8
SCALE = 1.0 / 32.0  # 1/sqrt(C)

# own q-tiles per parity, sorted by descending causal span
OWN_TILES = {0: [7, 4, 3, 0], 1: [6, 5, 2, 1]}
# active slot count per s-position p (p<4: own tiles, p>=4: partner tiles)
NP_ACT = [1, 2, 3, 4, 1, 2, 3, 4]

_CACHE = {}


def _perm(par):
    tiles = OWN_TILES[par] + OWN_TILES[1 - par]
    return np.concatenate([np.arange(k * P, (k + 1) * P) for k in tiles])


def _mask01_for(par):
    m = np.ones((P, NT, P), np.float32)
    tri = (np.arange(P)[:, None] <= np.arange(P)[None, :]).astype(np.float32)
    for p in range(4):
        m[:, p, :] = tri
    for j in range(4):
        if OWN_TILES[1 - par][j] > OWN_TILES[par][j]:
            m[:, 4 + j, :] = 0.0
    return m.astype(ml_dtypes.bfloat16)


def _build_nc():
    import concourse.bass as bass
    from concourse import bacc
    import concourse.mybir as mybir
    import concourse.tile as tile

    AF = mybir.ActivationFunctionType
    OP = mybir.AluOpType
    F32 = mybir.dt.float32
    BF = mybir.dt.bfloat16
    ts = bass.ts

    nc = bacc.Bacc()
    d_xT = nc.dram_tensor("xT", [C, T], F32, kind="ExternalInput")
    # blocked weights: see kernel() for the host-side layouts
    d_wq = nc.dram_tensor("wq", [NT, P, C], BF, kind="ExternalInput")
    d_wk = nc.dram_tensor("wk", [NT, P, C], BF, kind="ExternalInput")
    d_wv = nc.dram_tensor("wv", [C, C], BF, kind="ExternalInput")
    d_w1 = nc.dram_tensor("w1", [32, P, C], BF, kind="ExternalInput")
    d_w2 = nc.dram_tensor("w2", [2, 32, P, 512], BF, kind="ExternalInput")
    d_bq = nc.dram_tensor("bq", [P, NT], F32, kind="ExternalInput")
    d_bk = nc.dram_tensor("bk", [P, NT], F32, kind="ExternalInput")
    d_bvb = nc.dram_tensor("bvb", [P, C], F32, kind="ExternalInput")
    d_g1 = nc.dram_tensor("g1", [P, NT], F32, kind="ExternalInput")
    d_b1 = nc.dram_tensor("b1", [P, NT], F32, kind="ExternalInput")
    d_g2 = nc.dram_tensor("g2", [P, NT], F32, kind="ExternalInput")
    d_b2 = nc.dram_tensor("b2", [P, NT], F32, kind="ExternalInput")
    d_gf = nc.dram_tensor("gf", [P, NT], F32, kind="ExternalInput")
    d_bf = nc.dram_tensor("bf", [P, NT], F32, kind="ExternalInput")
    d_b1f = nc.dram_tensor("b1f", [P, 32], F32, kind="ExternalInput")
    d_b2f = nc.dram_tensor("b2f", [P, NT], F32, kind="ExternalInput")
    d_mask = nc.dram_tensor("mask", [P, NT, P], BF, kind="ExternalInput")
    d_out = nc.dram_tensor("outT", [C, OWN], F32, kind="ExternalOutput")

    with tile.TileContext(nc) as tc:
        const = tc.alloc_tile_pool(name="const", bufs=1)
        perst = tc.alloc_tile_pool(name="perst", bufs=1)
        work = tc.alloc_tile_pool(name="work", bufs=1)
        ps = tc.alloc_tile_pool(name="ps", bufs=8, space="PSUM")

        ones_bf = const.tile([P, P], BF)
        nc.vector.memset(ones_bf[:], 1.0)
        eps1 = const.tile([1, 1], F32)
        nc.vector.memset(eps1[:], 1e-5)

        def load_const(name, dram, shape, dt=F32):
            t = const.tile(shape, dt, name=name)
            nc.sync.dma_start(out=t[:], in_=dram[:])
            return t

        xt = work.tile([P, NT, T], F32, tag="A32", name="xt")
        sb_wv = work.tile([P, NT, C], BF, tag="B16", name="sb_wv")
        for c in range(NT):
            nc.sync.dma_start(out=xt[:, c, :], in_=d_xT[ts(c, P), :])
        for c in range(NT):
            nc.sync.dma_start(out=sb_wv[:, c, :], in_=d_wv[ts(c, P), :])

        sb_bq = load_const("sb_bq", d_bq, [P, NT])
        sb_bk = load_const("sb_bk", d_bk, [P, NT])
        sb_bvb = load_const("sb_bvb", d_bvb, [P, C])
        sb_g1 = load_const("sb_g1", d_g1, [P, NT])
        sb_b1 = load_const("sb_b1", d_b1, [P, NT])
        sb_g2 = load_const("sb_g2", d_g2, [P, NT])
        sb_b2 = load_const("sb_b2", d_b2, [P, NT])
        sb_gf = load_const("sb_gf", d_gf, [P, NT])
        sb_bf = load_const("sb_bf", d_bf, [P, NT])
        sb_b1f = load_const("sb_b1f", d_b1f, [P, 32])
        sb_b2f = load_const("sb_b2f", d_b2f, [P, NT])
        sb_mask = load_const("sb_mask", d_mask, [P, NT, P], BF)

        # persistent activations
        hbf = perst.tile([P, NT, T], BF)
        kT = perst.tile([P, NT, T], BF)
        vT = perst.tile([P, NT, C], BF)
        qT = perst.tile([P, NT, OWN], BF)
        x3 = perst.tile([P, NT, OWN], F32)

        def ln_stat(src_ap, cols, sum_ps, sq_ps, first, last, pfx):
            xb = work.tile([P, 512], BF, tag="xb", bufs=2, name=f"{pfx}xb")[:, 0:cols]
            sq = work.tile([P, 512], BF, tag="sq", bufs=2, name=f"{pfx}sqt")[:, 0:cols]
            nc.scalar.activation(xb[:], src_ap, AF.Copy)
            nc.scalar.activation(sq[:], src_ap, AF.Square)
            nc.tensor.matmul(sum_ps[:], ones_bf[:, 0:1], xb[:], start=first, stop=last)
            nc.tensor.matmul(sq_ps[:], ones_bf[:, 0:1], sq[:], start=first, stop=last)

        def ln_finish(sum_ps, sq_ps, src, cols, g_ap, b_ap, outs, pfx):
            mu = work.tile([1, 512], F32, tag="mu", bufs=2, name=f"{pfx}mu")[:, 0:cols]
            t2 = work.tile([1, 512], F32, tag="t2", bufs=2, name=f"{pfx}t2")[:, 0:cols]
            rstd = work.tile([1, 512], F32, tag="rstd", bufs=2, name=f"{pfx}rstd")[:, 0:cols]
            nc.scalar.activation(mu[:], sum_ps[:], AF.Copy, scale=1.0 / C)
            nc.scalar.activation(t2[:], sq_ps[:], AF.Copy, scale=1.0 / C)
            nc.vector.tensor_mul(rstd[:], mu[:], mu[:])
            nc.vector.tensor_sub(t2[:], t2[:], rstd[:])
            nc.scalar.activation(t2[:], t2[:], AF.Sqrt, bias=eps1[:])
            nc.vector.reciprocal_approx_fast(out=rstd[:], in_=t2[:])
            mu_bf = work.tile([1, 512], BF, tag="mubf", bufs=2, name=f"{pfx}mubf")[:, 0:cols]
            rs_bf = work.tile([1, 512], BF, tag="rsbf", bufs=2, name=f"{pfx}rsbf")[:, 0:cols]
            nc.scalar.activation(mu_bf[:], mu[:], AF.Copy)
            nc.scalar.activation(rs_bf[:], rstd[:], AF.Copy)
            mu_bc = ps.tile([P, cols], F32, tag="bank", name=f"{pfx}mubc")
            rs_bc = ps.tile([P, cols], F32, tag="bank", name=f"{pfx}rsbc")
            nc.tensor.matmul(mu_bc[:], ones_bf[0:1, 0:P], mu_bf[0:1, :],
                             start=True, stop=True)
            nc.tensor.matmul(rs_bc[:], ones_bf[0:1, 0:P], rs_bf[0:1, :],
                             start=True, stop=True)
            for c in range(NT):
                s = src(c)
                t0 = work.tile([P, 512], F32, tag="t0", bufs=2, name=f"{pfx}t0")[:, 0:cols]
                nc.vector.tensor_sub(t0[:], s, mu_bc[:])
                nc.vector.tensor_mul(t0[:], t0[:], rs_bc[:])
                for dst in outs(c):
                    nc.vector.tensor_scalar(
                        out=dst, in0=t0[:],
                        scalar1=g_ap(c), scalar2=b_ap(c),
                        op0=OP.mult, op1=OP.add)

        def layernorm_T(src, cols, g_ap, b_ap, outs, pfx):
            sum_ps = ps.tile([1, cols], F32, tag="bank", name=f"{pfx}sum")
            sq_ps = ps.tile([1, cols], F32, tag="bank", name=f"{pfx}sq")
            for c in range(NT):
                ln_stat(src(c), cols, sum_ps, sq_ps, c == 0, c == NT - 1, pfx)
            ln_finish(sum_ps, sq_ps, src, cols, g_ap, b_ap, outs, pfx)

        # ---------------- Phase 1: LN1 (split halves) + QKV projections
        # lo half (own tokens) first so q/v projections can start early
        for h2 in range(2):
            cs = slice(h2 * 512, (h2 + 1) * 512)
            layernorm_T(lambda c: xt[:, c, cs], 512,
                        lambda c: sb_g1[:, c:c + 1], lambda c: sb_b1[:, c:c + 1],
                        lambda c: [hbf[:, c, cs]], pfx=f"ln1{h2}")
            # v token-major for this half's token tiles
            for t in range(h2 * 4, h2 * 4 + 4):
                for vh in range(2):
                    v_ps = ps.tile([P, 512], F32, tag="bank", name="v_ps")
                    for c in range(NT):
                        nc.tensor.matmul(v_ps[:], hbf[:, c, ts(t, P)],
                                         sb_wv[:, c, ts(vh, 512)],
                                         start=(c == 0), stop=(c == NT - 1))
                    nc.vector.tensor_add(vT[:, t, ts(vh, 512)], v_ps[:],
                                         sb_bvb[:, ts(vh, 512)])
            if h2 == 0:
                # qT[d, own tokens]
                for hp in range(NT):
                    wq_t = work.tile([P, C], BF, tag="W", bufs=4, name="wq_t")
                    nc.sync.dma_start(out=wq_t[:], in_=d_wq[hp])
                    q_ps = ps.tile([P, OWN], F32, tag="bank", name="q_ps")
                    for c in range(NT):
                        nc.tensor.matmul(q_ps[:], wq_t[:, ts(c, P)], hbf[:, c, 0:OWN],
                                         start=(c == 0), stop=(c == NT - 1))
                    nc.vector.tensor_scalar(out=qT[:, hp, :], in0=q_ps[:],
                                            scalar1=sb_bq[:, hp:hp + 1], scalar2=None,
                                            op0=OP.add)
        # ---------------- Phase 2: k-proj + attention, interleaved per hp
        # (dense k-proj chains keep the PE activity monitor warm through the
        # sparse attention stretches); LN2 stats stream in as x2[hp] lands.
        x2 = work.tile([P, NT, OWN], F32, tag="A32", name="x2")
        ln2_sum = ps.tile([1, OWN], F32, tag="bank", name="ln2sum")
        ln2_sq = ps.tile([1, OWN], F32, tag="bank", name="ln2sq")
        CUM = [0, 128, 384, 768, 1280, 1408, 1664, 2048]  # packed exp offsets
        for hp in range(NT):
            wk_t = work.tile([P, C], BF, tag="W", bufs=4, name="wk_t")
            nc.sync.dma_start(out=wk_t[:], in_=d_wk[hp])
            for h2 in range(2):
                k_ps = ps.tile([P, 512], F32, tag="bank", name="k_ps")
                for c in range(NT):
                    nc.tensor.matmul(k_ps[:], wk_t[:, ts(c, P)],
                                     hbf[:, c, ts(h2, 512)],
                                     start=(c == 0), stop=(c == NT - 1))
                nc.vector.tensor_scalar(out=kT[:, hp, ts(h2, 512)], in0=k_ps[:],
                                        scalar1=sb_bk[:, hp:hp + 1], scalar2=None,
                                        op0=OP.add)
            ex_e = work.tile([P, 2560], BF, tag="exe", bufs=2, name="ex_e")
            ex_o = work.tile([P, 2560], BF, tag="exo", bufs=2, name="ex_o")
            for p in range(NT):
                w = NP_ACT[p] * P
                po = CUM[p]
                st_e = ps.tile([P, OWN], F32, tag="bank", name="st_e")
                st_o = ps.tile([P, OWN], F32, tag="bank", name="st_o")
                nc.tensor.matmul(st_e[:, 0:w], kT[0:64, hp, ts(p, P)],
                                 qT[0:64, hp, 0:w], start=True, stop=True)
                nc.tensor.matmul(st_o[:, 0:w], kT[64:128, hp, ts(p, P)],
                                 qT[64:128, hp, 0:w], start=True, stop=True)
                nc.scalar.activation(ex_e[:, po:po + w], st_e[:, 0:w], AF.Exp,
                                     scale=SCALE)
                nc.scalar.activation(ex_o[:, po:po + w], st_o[:, 0:w], AF.Exp,
                                     scale=SCALE)
                im = p % 4
                mw = slice(po + im * P, po + (im + 1) * P)
                nc.vector.tensor_mul(ex_e[:, mw], ex_e[:, mw], sb_mask[:, p, :])
                nc.vector.tensor_mul(ex_o[:, mw], ex_o[:, mw], sb_mask[:, p, :])
            gat = work.tile([P, OWN], F32, tag="gat", bufs=2, name="gat")
            rs = work.tile([P, OWN], F32, tag="rs", bufs=2, name="rs")
            for pair in range(2):
                av = []
                for i in range(pair * 2, pair * 2 + 2):
                    blocks = list(range(i, 4)) + list(range(4 + i, NT))
                    oTe = ps.tile([64, P], F32, tag="bank", name="oTe")
                    oTo = ps.tile([P, P], F32, tag="bank", name="oTo")
                    seE = ps.tile([64, P], F32, tag="bank", name="seE")
                    seO = ps.tile([P, P], F32, tag="bank", name="seO")
                    cs = ts(i, P)
                    for p in blocks:
                        first = (p == i)
                        last = (p == NT - 1)
                        pc = slice(CUM[p] + i * P, CUM[p] + (i + 1) * P)
                        nc.tensor.matmul(oTe[:, :], vT[:, p, hp * P:hp * P + 64],
                                         ex_e[:, pc], start=first, stop=last)
                        nc.tensor.matmul(oTo[64:128, :],
                                         vT[:, p, hp * P + 64:hp * P + 128],
                                         ex_o[:, pc], start=first, stop=last)
                        nc.tensor.matmul(seE[:, :], ones_bf[:, 0:64],
                                         ex_e[:, pc], start=first, stop=last)
                        nc.tensor.matmul(seO[64:128, :], ones_bf[:, 0:64],
                                         ex_o[:, pc], start=first, stop=last)
                    nc.vector.tensor_copy(gat[0:64, cs], seE[:, :])
                    nc.vector.tensor_copy(gat[64:128, cs], seO[64:128, :])
                    av.append((oTe, oTo))
                ps2 = slice(pair * 256, pair * 256 + 256)
                nc.vector.reciprocal_approx_fast(out=rs[:, ps2], in_=gat[:, ps2])
                for k2, (oTe, oTo) in enumerate(av):
                    i = pair * 2 + k2
                    cs = ts(i, P)
                    ot = work.tile([P, P], F32, tag="ot", bufs=2, name="ot")
                    nc.vector.tensor_mul(ot[0:64, :], oTe[:, :], rs[0:64, cs])
                    nc.vector.tensor_mul(ot[64:128, :], oTo[64:128, :], rs[64:128, cs])
                    # x2 = h + attn_out (residual uses post-LN h)
                    nc.vector.tensor_add(x2[:, hp, cs], ot[:, :],
                                         hbf[:, hp, i * P:(i + 1) * P])
            ln_stat(x2[:, hp, :], OWN, ln2_sum, ln2_sq, hp == 0, hp == NT - 1, "ln2")

        # ---------------- Phase 3: LN2, LNf, FFN ----------------
        hfb = work.tile([P, NT, OWN], BF, tag="C16", name="hfb")
        ln_finish(ln2_sum, ln2_sq, lambda c: x2[:, c, :], 512,
                  lambda c: sb_g2[:, c:c + 1], lambda c: sb_b2[:, c:c + 1],
                  lambda c: [x3[:, c, :]], pfx="ln2")
        for q2 in range(2):
            qs = slice(q2 * 256, (q2 + 1) * 256)
            layernorm_T(lambda c: x3[:, c, qs], 256,
                        lambda c: sb_gf[:, c:c + 1], lambda c: sb_bf[:, c:c + 1],
                        lambda c: [hfb[:, c, qs]], pfx=f"lnf{q2}")

        relu = work.tile([P, 32, OWN], BF, tag="A32", name="relu")
        for m in range(32):
            w1_t = work.tile([P, C], BF, tag="W", bufs=4, name="w1_t")
            nc.sync.dma_start(out=w1_t[:], in_=d_w1[m])
            f_ps = ps.tile([P, OWN], F32, tag="bank", name="f_ps")
            for c in range(NT):
                nc.tensor.matmul(f_ps[:], w1_t[:, ts(c, P)], hfb[:, c, :],
                                 start=(c == 0), stop=(c == NT - 1))
            nc.scalar.activation(relu[:, m, :], f_ps[:], AF.Relu,
                                 bias=sb_b1f[:, m:m + 1])

        for half in range(2):
            o_ps = [ps.tile([P, OWN], F32, tag="bank", name=f"o_ps{j}")
                    for j in range(4)]
            for m in range(32):
                w2t = work.tile([P, 512], BF, tag="W", bufs=4, name="w2t")
                nc.sync.dma_start(out=w2t[:], in_=d_w2[half, m])
                for j in range(4):
                    nc.tensor.matmul(o_ps[j][:], w2t[:, ts(j, P)], relu[:, m, :],
                                     start=(m == 0), stop=(m == 31))
            for j in range(4):
                co = half * 4 + j
                t0 = work.tile([P, OWN], F32, tag="fft", bufs=2, name="fft")
                nc.vector.scalar_tensor_tensor(
                    out=t0[:], in0=o_ps[j][:],
                    scalar=sb_b2f[:, co:co + 1], in1=x3[:, co, :],
                    op0=OP.add, op1=OP.add)
                out_sb = work.tile([P, OWN], F32, tag="osb", bufs=2, name="osb")
                nc.vector.tensor_add(out_sb[:], t0[:], hfb[:, co, :])
                nc.sync.dma_start(out=d_out[ts(co, P), :], in_=out_sb[:])

        ps.release()
        work.release()
        perst.release()
        const.release()

    nc.finalize()
    return nc


def _get_nc():
    if "nc" not in _CACHE:
        _CACHE["nc"] = _build_nc()
    return _CACHE["nc"]


def kernel(**inputs):
    from concourse.bass_utils import run_bass_kernel_spmd

    nc = _get_nc()
    bf16 = ml_dtypes.bfloat16

    f = {k: np.asarray(v, dtype=np.float32) for k, v in inputs.items()}
    x = f["x"]

    def stack_heads(w):  # [H, C, hd] -> [C, H*hd]
        return np.ascontiguousarray(w.transpose(1, 0, 2).reshape(C, C))

    def block_lhsT(w, nm):  # [C, nm*128] -> [nm, P, C] per-tile lhsT blocks
        return np.ascontiguousarray(
            w.reshape(NT, P, nm, P).transpose(2, 1, 0, 3).reshape(nm, P, NT * P))

    def part_scalar(v, n):  # [n*128] -> [128, n]
        return np.ascontiguousarray(v.reshape(-1).reshape(n, P).T)

    wq_full = stack_heads(f["Wq"])
    wk_full = stack_heads(f["Wk"])
    shared = {
        "wq": block_lhsT(wq_full, NT).astype(bf16),
        "wk": block_lhsT(wk_full, NT).astype(bf16),
        "wv": stack_heads(f["Wv"]).astype(bf16),
        "w1": block_lhsT(np.ascontiguousarray(f["W1"]), 32).astype(bf16),
        "w2": np.ascontiguousarray(
            f["W2"].reshape(32, P, 2, 512).transpose(2, 0, 1, 3)).astype(bf16),
        "bq": part_scalar(f["bq"], NT),
        "bk": part_scalar(f["bk"], NT),
        "bvb": np.ascontiguousarray(np.broadcast_to(f["bv"].reshape(-1), (P, C))),
        "g1": part_scalar(f["g1"], NT),
        "b1": part_scalar(f["b1"], NT),
        "g2": part_scalar(f["g2"], NT),
        "b2": part_scalar(f["b2"], NT),
        "gf": part_scalar(f["gf"], NT),
        "bf": part_scalar(f["bf"], NT),
        "b1f": part_scalar(f["b1f"], 32),
        "b2f": part_scalar(f["b2f"], NT),
    }
    masks = {par: _mask01_for(par) for par in (0, 1)}
    perms = {par: _perm(par) for par in (0, 1)}

    in_maps = []
    for core in range(N_CORES):
        b, par = core // 2, core % 2
        xT = np.ascontiguousarray(x[b].T[:, perms[par]])
        in_maps.append({**shared, "xT": xT, "mask": masks[par]})

    res = run_bass_kernel_spmd(nc, in_maps, list(range(N_CORES)))

    out = np.empty((B, T, C), np.float32)
    for core in range(N_CORES):
        b, par = core // 2, core % 2
        outT = res.results[core]["outT"]  # [C, OWN]
        out[b, perms[par][:OWN], :] = outT.T
    return out


# revision 12
# speedup vs baseline: 1.0477x; 1.0477x over previous
"""Trainium2 Bass kernel: single transformer decoder block
(B=4, T=1024, C=1024, H=16 heads, head_dim=64, FFN hidden=4096).

Sharding: sequence-parallel across 8 NeuronCores = 4 batches x 2
causally-balanced token halves.  No collectives: each core computes
LN1 + K/V projections for all 1024 tokens of its batch, and
attention + FFN for its own 512 tokens.  The host shards/permutes on
the way in and gathers/unpermutes on the way out.

On-chip everything runs in a transposed layout [C on partitions,
tokens on free dim]:
  - LN mean/var via ones-matmul partition reductions (bf16 operands,
    fp32 PSUM) + K=1 broadcast matmuls.
  - Scores are computed transposed (S^T = K_h^T-stationary x Q_h^T);
    causal masking is a multiplicative bf16 0/1 mask on exp(S^T); the
    softmax denominator comes from a ones-matmul accumulated alongside
    the A@V matmul.  No PE transposes, no max-subtraction (logits are
    bounded ~|0.8|).
  - Matmuls in bf16 with fp32 PSUM accumulation (rel err ~2e-3).
  - Every PSUM accumulation chain owns a whole bank (start=True resets
    accumulation state bank-wide, verified on HW).

Token permutation makes the kernel uniform across cores: own token
tiles (sorted by descending causal span) sit at positions 0..3,
partner tiles at 4..7, so even/odd cores share one instruction
stream; per-core causal differences live in the mask01 tensor.

Single long-lived SBUF/PSUM pools with tag-chained slot reuse: no
mid-kernel pool releases (each release is an all-engine barrier).
"""

import numpy as np
import ml_dtypes

B, T, C, H = 4, 1024, 1024, 16
HD = 64
FF = 4096
P = 128
NT = 8  # token tiles == C tiles
OWN = 512
N_CORES = 8
SCALE = 1.0 / 32.0  # 1/sqrt(C)

# own q-tiles per parity, sorted by descending causal span
OWN_TILES = {0: [7, 4, 3, 0], 1: [6, 5, 2, 1]}
# active slot count per s-position p (p<4: own tiles, p>=4: partner tiles)
NP_ACT = [1, 2, 3, 4, 1, 2, 3, 4]

_CACHE = {}


def _perm(par):
    tiles = OWN_TILES[par] + OWN_TILES[1 - par]
    return np.concatenate([np.arange(k * P, (k + 1) * P) for k in tiles])


def _mask01_for(par):
    m = np.ones((P, NT, P), np.float32)
    tri = (np.arange(P)[:, None] <= np.arange(P)[None, :]).astype(np.float32)
    for p in range(4):
        m[:, p, :] = tri
    for j in range(4):
        if OWN_TILES[1 - par][j] > OWN_TILES[par][j]:
            m[:, 4 + j, :] = 0.0
    return m.astype(ml_dtypes.bfloat16)


def _build_nc():
    import concourse.bass as bass
    from concourse import bacc
    import concourse.mybir as mybir
    import concourse.tile as tile

    AF = mybir.ActivationFunctionType
    OP = mybir.AluOpType
    F32 = mybir.dt.float32
    BF = mybir.dt.bfloat16
    ts = bass.ts

    nc = bacc.Bacc()
    d_xT = nc.dram_tensor("xT", [C, T], F32, kind="ExternalInput")
    # blocked weights: see kernel() for the host-side layouts
    d_wq = nc.dram_tensor("wq", [NT, P, C], BF, kind="ExternalInput")
    d_wk = nc.dram_tensor("wk", [NT, P, C], BF, kind="ExternalInput")
    d_wv = nc.dram_tensor("wv", [C, C], BF, kind="ExternalInput")
    d_w1 = nc.dram_tensor("w1", [32, P, C], BF, kind="ExternalInput")
    d_w2 = nc.dram_tensor("w2", [2, 32, P, 512], BF, kind="ExternalInput")
    d_bq = nc.dram_tensor("bq", [P, NT], F32, kind="ExternalInput")
    d_bk = nc.dram_tensor("bk", [P, NT], F32, kind="ExternalInput")
    d_bvb = nc.dram_tensor("bvb", [P, C], F32, kind="ExternalInput")
    d_g1 = nc.dram_tensor("g1", [P, NT], F32, kind="ExternalInput")
    d_b1 = nc.dram_tensor("b1", [P, NT], F32, kind="ExternalInput")
    d_g2 = nc.dram_tensor("g2", [P, NT], F32, kind="ExternalInput")
    d_b2 = nc.dram_tensor("b2", [P, NT], F32, kind="ExternalInput")
    d_gf = nc.dram_tensor("gf", [P, NT], F32, kind="ExternalInput")
    d_bf = nc.dram_tensor("bf", [P, NT], F32, kind="ExternalInput")
    d_b1f = nc.dram_tensor("b1f", [P, 32], F32, kind="ExternalInput")
    d_b2f = nc.dram_tensor("b2f", [P, NT], F32, kind="ExternalInput")
    d_mask = nc.dram_tensor("mask", [P, NT, P], BF, kind="ExternalInput")
    d_out = nc.dram_tensor("outT", [C, OWN], F32, kind="ExternalOutput")

    with tile.TileContext(nc) as tc:
        const = tc.alloc_tile_pool(name="const", bufs=1)
        perst = tc.alloc_tile_pool(name="perst", bufs=1)
        work = tc.alloc_tile_pool(name="work", bufs=1)
        ps = tc.alloc_tile_pool(name="ps", bufs=8, space="PSUM")

        ones_bf = const.tile([P, P], BF)
        nc.vector.memset(ones_bf[:], 1.0)
        eps1 = const.tile([1, 1], F32)
        nc.vector.memset(eps1[:], 1e-5)

        def load_const(name, dram, shape, dt=F32):
            t = const.tile(shape, dt, name=name)
            nc.sync.dma_start(out=t[:], in_=dram[:])
            return t

        xt = work.tile([P, NT, T], F32, tag="A32", name="xt")
        sb_wv = work.tile([P, NT, C], BF, tag="B16", name="sb_wv")
        for c in range(NT):
            nc.sync.dma_start(out=xt[:, c, :], in_=d_xT[ts(c, P), :])
        for c in range(NT):
            nc.sync.dma_start(out=sb_wv[:, c, :], in_=d_wv[ts(c, P), :])

        sb_bq = load_const("sb_bq", d_bq, [P, NT])
        sb_bk = load_const("sb_bk", d_bk, [P, NT])
        sb_bvb = load_const("sb_bvb", d_bvb, [P, C])
        sb_g1 = load_const("sb_g1", d_g1, [P, NT])
        sb_b1 = load_const("sb_b1", d_b1, [P, NT])
        sb_g2 = load_const("sb_g2", d_g2, [P, NT])
        sb_b2 = load_const("sb_b2", d_b2, [P, NT])
        sb_gf = load_const("sb_gf", d_gf, [P, NT])
        sb_bf = load_const("sb_bf", d_bf, [P, NT])
        sb_b1f = load_const("sb_b1f", d_b1f, [P, 32])
        sb_b2f = load_const("sb_b2f", d_b2f, [P, NT])
        sb_mask = load_const("sb_mask", d_mask, [P, NT, P], BF)

        # persistent activations
        hbf = perst.tile([P, NT, T], BF)
        kT = perst.tile([P, NT, T], BF)
        vT = perst.tile([P, NT, C], BF)
        qT = perst.tile([P, NT, OWN], BF)
        x3 = perst.tile([P, NT, OWN], F32)

        def ln_stat(src_ap, cols, sum_ps, sq_ps, first, last, pfx):
            xb = work.tile([P, 512], BF, tag="xb", bufs=2, name=f"{pfx}xb")[:, 0:cols]
            sq = work.tile([P, 512], BF, tag="sq", bufs=2, name=f"{pfx}sqt")[:, 0:cols]
            nc.scalar.activation(xb[:], src_ap, AF.Copy)
            nc.scalar.activation(sq[:], src_ap, AF.Square)
            nc.tensor.matmul(sum_ps[:], ones_bf[:, 0:1], xb[:], start=first, stop=last)
            nc.tensor.matmul(sq_ps[:], ones_bf[:, 0:1], sq[:], start=first, stop=last)

        def ln_finish(sum_ps, sq_ps, src, cols, g_ap, b_ap, outs, pfx):
            mu = work.tile([1, 512], F32, tag="mu", bufs=2, name=f"{pfx}mu")[:, 0:cols]
            t2 = work.tile([1, 512], F32, tag="t2", bufs=2, name=f"{pfx}t2")[:, 0:cols]
            rstd = work.tile([1, 512], F32, tag="rstd", bufs=2, name=f"{pfx}rstd")[:, 0:cols]
            nc.scalar.activation(mu[:], sum_ps[:], AF.Copy, scale=1.0 / C)
            nc.scalar.activation(t2[:], sq_ps[:], AF.Copy, scale=1.0 / C)
            nc.vector.tensor_mul(rstd[:], mu[:], mu[:])
            nc.vector.tensor_sub(t2[:], t2[:], rstd[:])
            nc.scalar.activation(t2[:], t2[:], AF.Sqrt, bias=eps1[:])
            nc.vector.reciprocal_approx_fast(out=rstd[:], in_=t2[:])
            mu_bf = work.tile([1, 512], BF, tag="mubf", bufs=2, name=f"{pfx}mubf")[:, 0:cols]
            rs_bf = work.tile([1, 512], BF, tag="rsbf", bufs=2, name=f"{pfx}rsbf")[:, 0:cols]
            nc.scalar.activation(mu_bf[:], mu[:], AF.Copy)
            nc.scalar.activation(rs_bf[:], rstd[:], AF.Copy)
            mu_bc = ps.tile([P, cols], F32, tag="bank", name=f"{pfx}mubc")
            rs_bc = ps.tile([P, cols], F32, tag="bank", name=f"{pfx}rsbc")
            nc.tensor.matmul(mu_bc[:], ones_bf[0:1, 0:P], mu_bf[0:1, :],
                             start=True, stop=True)
            nc.tensor.matmul(rs_bc[:], ones_bf[0:1, 0:P], rs_bf[0:1, :],
                             start=True, stop=True)
            for c in range(NT):
                s = src(c)
                t0 = work.tile([P, 512], F32, tag="t0", bufs=2, name=f"{pfx}t0")[:, 0:cols]
                nc.vector.tensor_sub(t0[:], s, mu_bc[:])
                nc.vector.tensor_mul(t0[:], t0[:], rs_bc[:])
                for dst in outs(c):
                    nc.vector.tensor_scalar(
                        out=dst, in0=t0[:],
                        scalar1=g_ap(c), scalar2=b_ap(c),
                        op0=OP.mult, op1=OP.add)

        def layernorm_T(src, cols, g_ap, b_ap, outs, pfx):
            sum_ps = ps.tile([1, cols], F32, tag="bank", name=f"{pfx}sum")
            sq_ps = ps.tile([1, cols], F32, tag="bank", name=f"{pfx}sq")
            for c in range(NT):
                ln_stat(src(c), cols, sum_ps, sq_ps, c == 0, c == NT - 1, pfx)
            ln_finish(sum_ps, sq_ps, src, cols, g_ap, b_ap, outs, pfx)

        # ---------------- Phase 1: LN1 (split halves) + QKV projections
        # lo half (own tokens) first so q/v projections can start early
        for h2 in range(2):
            cs = slice(h2 * 512, (h2 + 1) * 512)
            layernorm_T(lambda c: xt[:, c, cs], 512,
                        lambda c: sb_g1[:, c:c + 1], lambda c: sb_b1[:, c:c + 1],
                        lambda c: [hbf[:, c, cs]], pfx=f"ln1{h2}")
            # v token-major for this half's token tiles
            for t in range(h2 * 4, h2 * 4 + 4):
                for vh in range(2):
                    v_ps = ps.tile([P, 512], F32, tag="bank", name="v_ps")
                    for c in range(NT):
                        nc.tensor.matmul(v_ps[:], hbf[:, c, ts(t, P)],
                                         sb_wv[:, c, ts(vh, 512)],
                                         start=(c == 0), stop=(c == NT - 1))
                    nc.vector.tensor_add(vT[:, t, ts(vh, 512)], v_ps[:],
                                         sb_bvb[:, ts(vh, 512)])
            if h2 == 0:
                # qT[d, own tokens]
                for hp in range(NT):
                    wq_t = work.tile([P, C], BF, tag="W", bufs=4, name="wq_t")
                    nc.sync.dma_start(out=wq_t[:], in_=d_wq[hp])
                    q_ps = ps.tile([P, OWN], F32, tag="bank", name="q_ps")
                    for c in range(NT):
                        nc.tensor.matmul(q_ps[:], wq_t[:, ts(c, P)], hbf[:, c, 0:OWN],
                                         start=(c == 0), stop=(c == NT - 1))
                    nc.vector.tensor_scalar(out=qT[:, hp, :], in0=q_ps[:],
                                            scalar1=sb_bq[:, hp:hp + 1], scalar2=None,
                                            op0=OP.add)
        # ---------------- Phase 2: k-proj + attention, interleaved per hp
        # (dense k-proj chains keep the PE activity monitor warm through the
        # sparse attention stretches); LN2 stats stream in as x2[hp] lands.
        x2 = work.tile([P, NT, OWN], F32, tag="A32", name="x2")
        CUM = [0, 128, 384, 768, 1280, 1408, 1664, 2048]  # packed exp offsets
        for hp in range(NT):
            wk_t = work.tile([P, C], BF, tag="W", bufs=4, name="wk_t")
            nc.sync.dma_start(out=wk_t[:], in_=d_wk[hp])
            for h2 in range(2):
                k_ps = ps.tile([P, 512], F32, tag="bank", name="k_ps")
                for c in range(NT):
                    nc.tensor.matmul(k_ps[:], wk_t[:, ts(c, P)],
                                     hbf[:, c, ts(h2, 512)],
                                     start=(c == 0), stop=(c == NT - 1))
                nc.vector.tensor_scalar(out=kT[:, hp, ts(h2, 512)], in0=k_ps[:],
                                        scalar1=sb_bk[:, hp:hp + 1], scalar2=None,
                                        op0=OP.add)
            ex_e = work.tile([P, 2560], BF, tag="exe", bufs=2, name="ex_e")
            ex_o = work.tile([P, 2560], BF, tag="exo", bufs=2, name="ex_o")
            for p in range(NT):
                w = NP_ACT[p] * P
                po = CUM[p]
                st_e = ps.tile([P, OWN], F32, tag="bank", name="st_e")
                st_o = ps.tile([P, OWN], F32, tag="bank", name="st_o")
                nc.tensor.matmul(st_e[:, 0:w], kT[0:64, hp, ts(p, P)],
                                 qT[0:64, hp, 0:w], start=True, stop=True)
                nc.tensor.matmul(st_o[:, 0:w], kT[64:128, hp, ts(p, P)],
                                 qT[64:128, hp, 0:w], start=True, stop=True)
                nc.scalar.activation(ex_e[:, po:po + w], st_e[:, 0:w], AF.Exp,
                                     scale=SCALE)
                nc.scalar.activation(ex_o[:, po:po + w], st_o[:, 0:w], AF.Exp,
                                     scale=SCALE)
                im = p % 4
                mw = slice(po + im * P, po + (im + 1) * P)
                nc.vector.tensor_mul(ex_e[:, mw], ex_e[:, mw], sb_mask[:, p, :])
                nc.vector.tensor_mul(ex_o[:, mw], ex_o[:, mw], sb_mask[:, p, :])
            gat = work.tile([P, OWN], F32, tag="gat", bufs=2, name="gat")
            rs = work.tile([P, OWN], F32, tag="rs", bufs=2, name="rs")
            for pair in range(2):
                av = []
                for i in range(pair * 2, pair * 2 + 2):
                    blocks = list(range(i, 4)) + list(range(4 + i, NT))
                    oTe = ps.tile([64, P], F32, tag="bank", name="oTe")
                    oTo = ps.tile([P, P], F32, tag="bank", name="oTo")
                    seE = ps.tile([64, P], F32, tag="bank", name="seE")
                    seO = ps.tile([P, P], F32, tag="bank", name="seO")
                    cs = ts(i, P)
                    for p in blocks:
                        first = (p == i)
                        last = (p == NT - 1)
                        pc = slice(CUM[p] + i * P, CUM[p] + (i + 1) * P)
                        nc.tensor.matmul(oTe[:, :], vT[:, p, hp * P:hp * P + 64],
                                         ex_e[:, pc], start=first, stop=last)
                        nc.tensor.matmul(oTo[64:128, :],
                                         vT[:, p, hp * P + 64:hp * P + 128],
                                         ex_o[:, pc], start=first, stop=last)
                        nc.tensor.matmul(seE[:, :], ones_bf[:, 0:64],
                                         ex_e[:, pc], start=first, stop=last)
                        nc.tensor.matmul(seO[64:128, :], ones_bf[:, 0:64],
                                         ex_o[:, pc], start=first, stop=last)
                    nc.vector.tensor_copy(gat[0:64, cs], seE[:, :])
                    nc.vector.tensor_copy(gat[64:128, cs], seO[64:128, :])
                    av.append((oTe, oTo))
                ps2 = slice(pair * 256, pair * 256 + 256)
                nc.vector.reciprocal_approx_fast(out=rs[:, ps2], in_=gat[:, ps2])
                for k2, (oTe, oTo) in enumerate(av):
                    i = pair * 2 + k2
                    cs = ts(i, P)
                    ot = work.tile([P, P], F32, tag="ot", bufs=2, name="ot")
                    nc.vector.tensor_mul(ot[0:64, :], oTe[:, :], rs[0:64, cs])
                    nc.vector.tensor_mul(ot[64:128, :], oTo[64:128, :], rs[64:128, cs])
                    # x2 = h + attn_out (residual uses post-LN h)
                    nc.vector.tensor_add(x2[:, hp, cs], ot[:, :],
                                         hbf[:, hp, i * P:(i + 1) * P])

        # ---------------- Phase 3: LN2, LNf, FFN ----------------
        hfb = work.tile([P, NT, OWN], BF, tag="C16", name="hfb")
        layernorm_T(lambda c: x2[:, c, :], 512,
                    lambda c: sb_g2[:, c:c + 1], lambda c: sb_b2[:, c:c + 1],
                    lambda c: [x3[:, c, :]], pfx="ln2")
        for q2 in range(2):
            qs = slice(q2 * 256, (q2 + 1) * 256)
            layernorm_T(lambda c: x3[:, c, qs], 256,
                        lambda c: sb_gf[:, c:c + 1], lambda c: sb_bf[:, c:c + 1],
                        lambda c: [hfb[:, c, qs]], pfx=f"lnf{q2}")

        relu = work.tile([P, 32, OWN], BF, tag="A32", name="relu")
        for m in range(32):
            w1_t = work.tile([P, C], BF, tag="W", bufs=4, name="w1_t")
            nc.sync.dma_start(out=w1_t[:], in_=d_w1[m])
            f_ps = ps.tile([P, OWN], F32, tag="bank", name="f_ps")
            for c in range(NT):
                nc.tensor.matmul(f_ps[:], w1_t[:, ts(c, P)], hfb[:, c, :],
                                 start=(c == 0), stop=(c == NT - 1))
            nc.scalar.activation(relu[:, m, :], f_ps[:], AF.Relu,
                                 bias=sb_b1f[:, m:m + 1])

        for half in range(2):
            o_ps = [ps.tile([P, OWN], F32, tag="bank", name=f"o_ps{j}")
                    for j in range(4)]
            for m in range(32):
                w2t = work.tile([P, 512], BF, tag="W", bufs=4, name="w2t")
                nc.sync.dma_start(out=w2t[:], in_=d_w2[half, m])
                for j in range(4):
                    nc.tensor.matmul(o_ps[j][:], w2t[:, ts(j, P)], relu[:, m, :],
                                     start=(m == 0), stop=(m == 31))
            for j in range(4):
                co = half * 4 + j
                t0 = work.tile([P, OWN], F32, tag="fft", bufs=2, name="fft")
                nc.vector.scalar_tensor_tensor(
                    out=t0[:], in0=o_ps[j][:],
                    scalar=sb_b2f[:, co:co + 1], in1=x3[:, co, :],
                    op0=OP.add, op1=OP.add)
                out_sb = work.tile([P, OWN], F32, tag="osb", bufs=2, name="osb")
                nc.vector.tensor_add(out_sb[:], t0[:], hfb[:, co, :])
                nc.sync.dma_start(out=d_out[ts(co, P), :], in_=out_sb[:])

        ps.release()
        work.release()
        perst.release()
        const.release()

    nc.finalize()
    return nc


def _get_nc():
    if "nc" not in _CACHE:
        _CACHE["nc"] = _build_nc()
    return _CACHE["nc"]


def kernel(**inputs):
    from concourse.bass_utils import run_bass_kernel_spmd

    nc = _get_nc()
    bf16 = ml_dtypes.bfloat16

    f = {k: np.asarray(v, dtype=np.float32) for k, v in inputs.items()}
    x = f["x"]

    def stack_heads(w):  # [H, C, hd] -> [C, H*hd]
        return np.ascontiguousarray(w.transpose(1, 0, 2).reshape(C, C))

    def block_lhsT(w, nm):  # [C, nm*128] -> [nm, P, C] per-tile lhsT blocks
        return np.ascontiguousarray(
            w.reshape(NT, P, nm, P).transpose(2, 1, 0, 3).reshape(nm, P, NT * P))

    def part_scalar(v, n):  # [n*128] -> [128, n]
        return np.ascontiguousarray(v.reshape(-1).reshape(n, P).T)

    wq_full = stack_heads(f["Wq"])
    wk_full = stack_heads(f["Wk"])
    shared = {
        "wq": block_lhsT(wq_full, NT).astype(bf16),
        "wk": block_lhsT(wk_full, NT).astype(bf16),
        "wv": stack_heads(f["Wv"]).astype(bf16),
        "w1": block_lhsT(np.ascontiguousarray(f["W1"]), 32).astype(bf16),
        "w2": np.ascontiguousarray(
            f["W2"].reshape(32, P, 2, 512).transpose(2, 0, 1, 3)).astype(bf16),
        "bq": part_scalar(f["bq"], NT),
        "bk": part_scalar(f["bk"], NT),
        "bvb": np.ascontiguousarray(np.broadcast_to(f["bv"].reshape(-1), (P, C))),
        "g1": part_scalar(f["g1"], NT),
        "b1": part_scalar(f["b1"], NT),
        "g2": part_scalar(f["g2"], NT),
        "b2": part_scalar(f["b2"], NT),
        "gf": part_scalar(f["gf"], NT),
        "bf": part_scalar(f["bf"], NT),
        "b1f": part_scalar(f["b1f"], 32),
        "b2f": part_scalar(f["b2f"], NT),
    }
    masks = {par: _mask01_for(par) for par in (0, 1)}
    perms = {par: _perm(par) for par in (0, 1)}

    in_maps = []
    for core in range(N_CORES):
        b, par = core // 2, core % 2
        xT = np.ascontiguousarray(x[b].T[:, perms[par]])
        in_maps.append({**shared, "xT": xT, "mask": masks[par]})

    res = run_bass_kernel_spmd(nc, in_maps, list(range(N_CORES)))

    out = np.empty((B, T, C), np.float32)
    for core in range(N_CORES):
        b, par = core // 2, core % 2
        outT = res.results[core]["outT"]  # [C, OWN]
        out[b, perms[par][:OWN], :] = outT.T
    return out
